# revision 34
# baseline (speedup 1.0000x reference)
"""Trainium2 Bass kernel for nn_Model_34316788695805 (ragged_sequence).

Model: per-token char-level encoder GRU (C=8 steps) -> decoder GRU
(F=32 steps, teacher forced) -> vocab projection scores.

Sharding: token-parallel over 8 NeuronCores (32 tokens/core).  Each core
runs the full enc+dec GRU for its tokens and the full vocab projection,
producing a contiguous [1024, 10000] slab of the output.  No collectives;
the host concatenates the slabs.

Device layout: hidden dim H=1024 lives on partitions (8 chunks of 128),
tokens on the free axis.  GRU gate matmuls keep W_hh^T stationary
(bf16, FWL) and stream h (bf16).  Gate arithmetic is fp32 on DVE/ACT.
The vocab projection is a single batched matmul at the end over all
32 steps (lhsT = transposed hidden states, rhs = streamed out_W^T slabs).

Host-side work is limited to sharding/layout prep: embedding gathers,
mean-pooling of h0, weight transposes/casts, and EOS-freeze fixup
(a no-op for the generated inputs, which contain no EOS).
"""

import numpy as np
import ml_dtypes
from contextlib import ExitStack

import concourse.bass as bass
import concourse.mybir as mybir
import concourse.tile as tile
from concourse import bacc
from concourse.bass_utils import run_bass_kernel_spmd

# Problem constants (hardcoded per spec)
T, F, C, V, H, E, S = 256, 32, 8, 10000, 1024, 256, 512
PAD, BOS, EOS = 0, 1, 2
NCORES = 8
TC = T // NCORES          # 32 tokens per core
TS = TC * F               # 1024 (token,step) pairs per core
KH = H // 128             # 8 k-chunks of hidden
KE = E // 128             # 2 k-chunks of embedding
MG = H // 128             # 8 m-chunks per gate
VCH = 512                 # vocab chunk (one PSUM bank of fp32)
NV = (V + VCH - 1) // VCH  # 20 chunks
VPAD = NV * VCH           # 10240

F32 = mybir.dt.float32
BF16 = mybir.dt.bfloat16
AF = mybir.ActivationFunctionType
npbf16 = ml_dtypes.bfloat16

_CACHE = {}


def _to_lhsT_layout(w):
    """[M, K] weight -> [128, K//128, M] array so that
    arr[p, k, m] = w[m, 128*k + p]; lhsT tile (k, m0) = arr[:, k, m0:m0+128]."""
    M, K = w.shape
    return np.ascontiguousarray(w.T.reshape(K // 128, 128, M).transpose(1, 0, 2))


def _cols_layout(x):
    """[N, K] -> [128, K//128, N]: arr[p, k, n] = x[n, 128*k + p] (rhs/moving)."""
    N, K = x.shape
    return np.ascontiguousarray(x.T.reshape(K // 128, 128, N).transpose(1, 0, 2))


def _build_program(flags):
    """Build + compile the Bacc/Tile program. flags: (gib_enc, ghn_enc,
    gib_dec, ghn_dec, outb) nonzero-bias booleans."""
    has_gib_enc, has_ghn_enc, has_gib_dec, has_ghn_dec, has_outb = flags

    nc = bacc.Bacc(
        "TRN2",
        target_bir_lowering=False,
        debug=False,
        enable_asserts=False,
        num_devices=NCORES,
    )

    # ---- DRAM I/O ----
    d_h0 = nc.dram_tensor("h0T", [128, KH, TC], F32, kind="ExternalInput").ap()
    d_xenc = nc.dram_tensor("xencT", [128, KE, C * TC], BF16, kind="ExternalInput").ap()
    d_xdec = nc.dram_tensor("xdecT", [128, KE, TS], BF16, kind="ExternalInput").ap()
    d_whh_e = nc.dram_tensor("whhTe", [128, KH, 3 * H], BF16, kind="ExternalInput").ap()
    d_whh_d = nc.dram_tensor("whhTd", [128, KH, 3 * H], BF16, kind="ExternalInput").ap()
    d_wih_e = nc.dram_tensor("wihTe", [128, KE, 3 * H], BF16, kind="ExternalInput").ap()
    d_wih_d = nc.dram_tensor("wihTd", [128, KE, 3 * H], BF16, kind="ExternalInput").ap()
    d_ow = nc.dram_tensor("owT", [NV, 128, KH, VCH], BF16, kind="ExternalInput").ap()
    d_gib_e = d_gib_d = d_ghn_e = d_ghn_d = d_outb = None
    if has_gib_enc:
        d_gib_e = nc.dram_tensor("gibE", [128, 24], F32, kind="ExternalInput").ap()
    if has_gib_dec:
        d_gib_d = nc.dram_tensor("gibD", [128, 24], F32, kind="ExternalInput").ap()
    if has_ghn_enc:
        d_ghn_e = nc.dram_tensor("ghnE", [128, MG], F32, kind="ExternalInput").ap()
    if has_ghn_dec:
        d_ghn_d = nc.dram_tensor("ghnD", [128, MG], F32, kind="ExternalInput").ap()
    if has_outb:
        d_outb = nc.dram_tensor("outb", [1, VPAD], BF16, kind="ExternalInput").ap()
    d_scores = nc.dram_tensor("scores", [TS, V], F32, kind="ExternalOutput").ap()

    with tile.TileContext(nc) as tc, ExitStack() as ctx:
        wpool = ctx.enter_context(tc.tile_pool(name="weights", bufs=1))
        whpool = ctx.enter_context(tc.tile_pool(name="whh", bufs=2))
        wipool = ctx.enter_context(tc.tile_pool(name="wih", bufs=1))
        gipool = ctx.enter_context(tc.tile_pool(name="gi", bufs=2))
        hpool = ctx.enter_context(tc.tile_pool(name="h", bufs=2))
        gpool = ctx.enter_context(tc.tile_pool(name="gates", bufs=1))
        spool = ctx.enter_context(tc.tile_pool(name="slab", bufs=4))
        stpool = ctx.enter_context(tc.tile_pool(name="staging", bufs=3))
        ps_gh = ctx.enter_context(tc.tile_pool(name="ps_gh", bufs=1, space="PSUM"))
        ps_gi = ctx.enter_context(tc.tile_pool(name="ps_gi", bufs=2, space="PSUM"))
        ps_sc = ctx.enter_context(tc.tile_pool(name="ps_sc", bufs=3, space="PSUM"))

        # ---- resident weights / inputs (small/urgent first: one FIFO ring) ----
        xenc = wpool.tile([128, KE, C * TC], BF16, tag="xenc")
        nc.sync.dma_start(xenc[:], d_xenc)
        h_f = hpool.tile([128, KH, TC], F32, tag="hf")
        nc.sync.dma_start(h_f[:], d_h0)
        h_b = hpool.tile([128, KH, TC], BF16, tag="hb")
        nc.vector.tensor_copy(h_b[:], h_f[:])
        wih_e = wipool.tile([128, KE, 3 * H], BF16, tag="wih")
        nc.sync.dma_start(wih_e[:], d_wih_e)
        # gate-split DMAs: enc step 0's r-gate matmuls start after 1/3 loads
        whh_e = whpool.tile([128, KH, 3 * H], BF16, tag="whh")
        for g in (0, 2, 1):
            nc.sync.dma_start(
                whh_e[:, :, g * H : (g + 1) * H], d_whh_e[:, :, g * H : (g + 1) * H]
            )
        xdec = wpool.tile([128, KE, TS], BF16, tag="xdec")
        nc.sync.dma_start(xdec[:], d_xdec)
        # hidden-state history (step-major columns: ts = s*TC + t), bf16;
        # the output DMA access pattern restores token-major row order
        hstT = wpool.tile([128, KH, F, TC], BF16, tag="hstT")

        gib_e = gib_d = ghn_e = ghn_d = None
        if has_gib_enc:
            gib_e = wpool.tile([128, 24], F32, tag="gib_e")
            nc.sync.dma_start(gib_e[:], d_gib_e)
        if has_gib_dec:
            gib_d = wpool.tile([128, 24], F32, tag="gib_d")
            nc.sync.dma_start(gib_d[:], d_gib_d)
        if has_ghn_enc:
            ghn_e = wpool.tile([128, MG], F32, tag="ghn_e")
            nc.sync.dma_start(ghn_e[:], d_ghn_e)
        if has_ghn_dec:
            ghn_d = wpool.tile([128, MG], F32, tag="ghn_d")
            nc.sync.dma_start(ghn_d[:], d_ghn_d)
        ones_row = None
        if has_outb:
            ones_row = wpool.tile([1, 128], BF16, tag="ones")
            nc.vector.memset(ones_row[:], 1.0)
        outb_sb = None
        if has_outb:
            outb_sb = wpool.tile([1, VPAD], BF16, tag="outb")
            nc.sync.dma_start(outb_sb[:], d_outb)

        def gi_batch(gi_tile, wih, x_ap, nsteps, gib):
            """gi[gate, mchunk, ts] = W_ih @ x (+ gate bias) for a block of
            steps. x_ap: [128, KE, nsteps*TC] bf16."""
            n = nsteps * TC
            for g in range(3):
                for j in range(MG):
                    m = g * H + j * 128
                    ps = ps_gi.tile([128, n], F32, tag="ps_gi")
                    for k in range(KE):
                        nc.tensor.matmul(
                            ps[:, :n],
                            wih[:, k, m : m + 128],
                            x_ap[:, k, :],
                            start=(k == 0),
                            stop=(k == KE - 1),
                        )
                    if gib is not None:
                        nc.scalar.activation(
                            gi_tile[:, g, j, :n], ps[:, :n], AF.Identity,
                            bias=gib[:, g * MG + j : g * MG + j + 1],
                        )
                    else:
                        nc.scalar.copy(gi_tile[:, g, j, :n], ps[:, :n])

        def gru_step(whh, gi_tile, s_in_chunk, ghn, hst_write_step=None):
            """One GRU step: h (h_f/h_b tiles, closed over) -> new h.
            gi_tile slice cols [s_in_chunk*TC, +TC]."""
            nonlocal h_f, h_b
            c0 = s_in_chunk * TC
            gh_r = ps_gh.tile([128, MG, TC], F32, tag="gh_r")
            gh_z = ps_gh.tile([128, MG, TC], F32, tag="gh_z")
            gh_n = ps_gh.tile([128, MG, TC], F32, tag="gh_n")
            # emission order r, n, z: the r/n gate chains overlap the
            # remaining matmuls; only the short z chain trails the step.
            for g, ps in ((0, gh_r), (2, gh_n), (1, gh_z)):
                for j in range(MG):
                    m = g * H + j * 128
                    for k in range(KH):
                        nc.tensor.matmul(
                            ps[:, j, :],
                            whh[:, k, m : m + 128],
                            h_b[:, k, :],
                            start=(k == 0),
                            stop=(k == KH - 1),
                        )
            gi_r = gi_tile[:, 0, :, c0 : c0 + TC]
            gi_z = gi_tile[:, 1, :, c0 : c0 + TC]
            gi_n = gi_tile[:, 2, :, c0 : c0 + TC]

            r_pre = gpool.tile([128, MG, TC], F32, tag="r_pre")
            nc.vector.tensor_add(r_pre[:], gi_r, gh_r[:])
            r = gpool.tile([128, MG, TC], F32, tag="r")
            nc.scalar.activation(r[:], r_pre[:], AF.Sigmoid)
            # n path
            if ghn is not None:
                ghn_sb = gpool.tile([128, MG, TC], F32, tag="ghn_sb")
                for j in range(MG):
                    nc.scalar.activation(
                        ghn_sb[:, j, :], gh_n[:, j, :], AF.Identity,
                        bias=ghn[:, j : j + 1],
                    )
                n_src = ghn_sb[:]
            else:
                n_src = gh_n[:]
            rn = gpool.tile([128, MG, TC], F32, tag="rn")
            nc.vector.tensor_mul(rn[:], r[:], n_src)
            n_pre = gpool.tile([128, MG, TC], F32, tag="n_pre")
            nc.vector.tensor_add(n_pre[:], rn[:], gi_n)
            n = gpool.tile([128, MG, TC], F32, tag="n")
            nc.scalar.activation(n[:], n_pre[:], AF.Tanh)
            # d = h - n (fp32 h keeps recurrence precision; off critical path)
            d = gpool.tile([128, MG, TC], F32, tag="d")
            nc.vector.tensor_sub(d[:], h_f[:], n[:])
            # z tail: z_pre -> sigmoid -> zd -> h (bf16 direct for the PE)
            z_pre = gpool.tile([128, MG, TC], F32, tag="z_pre")
            nc.vector.tensor_add(z_pre[:], gi_z, gh_z[:])
            z = gpool.tile([128, MG, TC], F32, tag="z")
            nc.scalar.activation(z[:], z_pre[:], AF.Sigmoid)
            zd = gpool.tile([128, MG, TC], F32, tag="zd")
            nc.vector.tensor_mul(zd[:], z[:], d[:])
            h_b = hpool.tile([128, KH, TC], BF16, tag="hb")
            nc.vector.tensor_add(h_b[:], n[:], zd[:])
            h_f = hpool.tile([128, KH, TC], F32, tag="hf")
            nc.vector.tensor_add(h_f[:], n[:], zd[:])
            if hst_write_step is not None:
                nc.scalar.copy(hstT[:, :, hst_write_step, :], h_b[:])

        # ---- encoder ----
        GCH = 4  # steps per gi chunk
        gi_t = gipool.tile([128, 3, MG, GCH * TC], F32, tag="gi")
        gi_batch(gi_t, wih_e, xenc[:, :, 0 : GCH * TC], GCH, gib_e)
        for s in range(C):
            g = s // GCH
            gru_step(whh_e, gi_t, s % GCH, ghn_e)
            if s == 1:
                # dec weights load during the encoder (DMA is idle here)
                wih_d = wipool.tile([128, KE, 3 * H], BF16, tag="wih")
                nc.sync.dma_start(wih_d[:], d_wih_d)
                whh_d = whpool.tile([128, KH, 3 * H], BF16, tag="whh")
                nc.sync.dma_start(whh_d[:], d_whh_d)
            if s % GCH == 0 and g + 1 < C // GCH:
                gi_next = gipool.tile([128, 3, MG, GCH * TC], F32, tag="gi")
                gi_batch(
                    gi_next, wih_e,
                    xenc[:, :, (g + 1) * GCH * TC : (g + 2) * GCH * TC],
                    GCH, gib_e,
                )
            if s % GCH == GCH - 1 and g + 1 < C // GCH:
                gi_t = gi_next

        # output rows are stored step-major (row = s*TC + t, contiguous
        # per block); the host reorders rows to token-major afterwards
        def scores_block(sb, c, slab, par):
            """Scores for step block sb (4 steps) x vocab chunk c."""
            ncols = min(VCH, V - c * VCH)
            ps = ps_sc.tile([128, VCH], F32, tag="ps_sc")
            for k in range(KH):
                nc.tensor.matmul(
                    ps[:],
                    hstT[:, k, 4 * sb : 4 * sb + 4, :],
                    slab[:, k, :],
                    start=(k == 0),
                    stop=False if has_outb else (k == KH - 1),
                )
            if has_outb:
                nc.tensor.matmul(
                    ps[:], ones_row[:], outb_sb[:, c * VCH : (c + 1) * VCH],
                    start=False, stop=True,
                )
            st = stpool.tile([128, VCH], F32, tag="st")
            if par % 2 == 0:
                nc.vector.tensor_copy(st[:], ps[:])
            else:
                nc.scalar.copy(st[:], ps[:])
            # stores on the ACT HWDGE ring; slab loads stay on SP's
            nc.scalar.dma_start(
                d_scores[128 * sb : 128 * (sb + 1), c * VCH : c * VCH + ncols],
                st[:, :ncols],
            )

        # ---- decoder (scores for the first vocab chunks fill step tails) ----
        from collections import deque

        N_INTER = 4
        inter_slabs = []
        for c in range(N_INTER):
            slab = spool.tile([128, KH, VCH], BF16, tag="slab")
            nc.sync.dma_start(slab[:], d_ow[c])
            inter_slabs.append(slab)

        pending = deque()
        gi_t = gipool.tile([128, 3, MG, GCH * TC], F32, tag="gi")
        gi_batch(gi_t, wih_d, xdec[:, :, 0 : GCH * TC], GCH, gib_d)
        for s in range(F):
            g = s // GCH
            gru_step(whh_d, gi_t, s % GCH, ghn_d, hst_write_step=s)
            if s % GCH == 0 and g + 1 < F // GCH:
                gi_next = gipool.tile([128, 3, MG, GCH * TC], F32, tag="gi")
                gi_batch(
                    gi_next, wih_d,
                    xdec[:, :, (g + 1) * GCH * TC : (g + 2) * GCH * TC],
                    GCH, gib_d,
                )
            if s % GCH == GCH - 1 and g + 1 < F // GCH:
                gi_t = gi_next
            # one scores block from an ALREADY-COMPLETE step block per tail
            if pending:
                sb, c = pending.popleft()
                scores_block(sb, c, inter_slabs[c], sb + c)
            if s % 4 == 3:
                pending.extend((s // 4, c) for c in range(N_INTER))
        for sb, c in pending:
            scores_block(sb, c, inter_slabs[c], sb + c)

        # ---- remaining vocab projection ----
        for c in range(N_INTER, NV):
            slab = spool.tile([128, KH, VCH], BF16, tag="slab")
            nc.sync.dma_start(slab[:], d_ow[c])
            for sb in range(F // 4):
                scores_block(sb, c, slab, sb + c)

    nc.compile()
    return nc


def _prep_inputs(token_ctx, char_emb_w, enc_W_ih, enc_W_hh, enc_b_ih, enc_b_hh,
                 dec_W_ih, dec_W_hh, dec_b_ih, dec_b_hh, out_W, out_b,
                 in_sent_token_chars, out_chars):
    """Host-side sharding/layout prep. Returns (in_maps, flags, fixup_info)."""
    tcarr = np.asarray(in_sent_token_chars)[0].reshape(T, C, 3)
    chars = tcarr[:, :, 2]
    xt = tcarr[:, :, 1]
    token_ctx = np.asarray(token_ctx)[0]          # [S, H]
    char_emb_w = np.asarray(char_emb_w)           # [V, E]
    out_chars = np.asarray(out_chars)[0]          # [1 + T*F]

    h0 = token_ctx[xt].mean(axis=1).astype(np.float32)      # [T, H]
    x_enc = char_emb_w[chars]                                # [T, C, E]
    gold = out_chars[1 : 1 + T * F].reshape(T, F)
    c0 = out_chars[0]
    c_in = np.concatenate(
        [np.full((T, 1), c0, dtype=gold.dtype), gold[:, :-1]], axis=1
    )                                                        # [T, F]
    x_dec = char_emb_w[c_in]                                 # [T, F, E]

    # shared (replicated) weight layouts
    whhTe = _to_lhsT_layout(np.asarray(enc_W_hh)).astype(npbf16)
    whhTd = _to_lhsT_layout(np.asarray(dec_W_hh)).astype(npbf16)
    wihTe = _to_lhsT_layout(np.asarray(enc_W_ih)).astype(npbf16)
    wihTd = _to_lhsT_layout(np.asarray(dec_W_ih)).astype(npbf16)
    owpad = np.zeros((VPAD, H), np.float32)
    owpad[:V] = np.asarray(out_W)
    owT = np.ascontiguousarray(
        owpad.reshape(NV, VCH, KH, 128).transpose(0, 3, 2, 1)
    ).astype(npbf16)                                          # [NV,128,KH,VCH]

    def gate_bias(b_ih, b_hh):
        b_ih = np.asarray(b_ih); b_hh = np.asarray(b_hh)
        gib = np.concatenate([b_ih[: 2 * H] + b_hh[: 2 * H], b_ih[2 * H :]])
        ghn = b_hh[2 * H :]
        gib_l = np.ascontiguousarray(gib.reshape(24, 128).T).astype(np.float32)
        ghn_l = np.ascontiguousarray(ghn.reshape(MG, 128).T).astype(np.float32)
        return gib_l, ghn_l, bool(np.any(gib)), bool(np.any(ghn))

    gibE, ghnE, has_gib_e, has_ghn_e = gate_bias(enc_b_ih, enc_b_hh)
    gibD, ghnD, has_gib_d, has_ghn_d = gate_bias(dec_b_ih, dec_b_hh)
    out_b = np.asarray(out_b)
    has_outb = bool(np.any(out_b))
    outb_pad = np.zeros((1, VPAD), npbf16)
    outb_pad[0, :V] = out_b.astype(npbf16)

    flags = (has_gib_e, has_ghn_e, has_gib_d, has_ghn_d, has_outb)

    in_maps = []
    for ci in range(NCORES):
        sl = slice(ci * TC, (ci + 1) * TC)
        h0T = np.ascontiguousarray(
            h0[sl].T.reshape(KH, 128, TC).transpose(1, 0, 2)
        )
        # enc ts = c*TC + t (step-major)
        xe = x_enc[sl].transpose(1, 0, 2).reshape(C * TC, E).astype(np.float32)
        xencT = _cols_layout(xe).astype(npbf16)
        # dec ts = s*TC + t (step-major)
        xd = x_dec[sl].transpose(1, 0, 2).reshape(TS, E).astype(np.float32)
        xdecT = _cols_layout(xd).astype(npbf16)
        m = {
            "h0T": h0T, "xencT": xencT, "xdecT": xdecT,
            "whhTe": whhTe, "whhTd": whhTd, "wihTe": wihTe, "wihTd": wihTd,
            "owT": owT,
        }
        if has_gib_e: m["gibE"] = gibE
        if has_gib_d: m["gibD"] = gibD
        if has_ghn_e: m["ghnE"] = ghnE
        if has_ghn_d: m["ghnD"] = ghnD
        if has_outb: m["outb"] = outb_pad
        in_maps.append(m)

    return in_maps, flags, (gold, c0)


def _eos_fixup(scores, gold, c0):
    """Apply the reference's EOS freeze/pad semantics on the host.
    scores: [T, F, V] (modified in place)."""
    if c0 != EOS and not np.any(gold == EOS):
        return scores
    done0 = c0 == EOS
    for t in range(T):
        hits = np.nonzero(gold[t] == EOS)[0]
        if done0:
            first_done = 0
        elif len(hits):
            first_done = int(hits[0]) + 1
        else:
            continue
        if first_done == 0:
            scores[t, :, :] = 0.0
        elif first_done < F:
            scores[t, first_done:, :] = scores[t, first_done - 1, :]
    return scores


def kernel(**inputs) -> np.ndarray:
    assert int(inputs["max_tokens"]) == T
    assert int(inputs["max_form_len"]) == F
    assert int(inputs["use_teacher_forcing"]) == 1

    in_maps, flags, (gold, c0) = _prep_inputs(
        inputs["token_ctx"], inputs["char_emb_w"],
        inputs["enc_W_ih"], inputs["enc_W_hh"], inputs["enc_b_ih"], inputs["enc_b_hh"],
        inputs["dec_W_ih"], inputs["dec_W_hh"], inputs["dec_b_ih"], inputs["dec_b_hh"],
        inputs["out_W"], inputs["out_b"],
        inputs["in_sent_token_chars"], inputs["out_chars"],
    )

    if flags not in _CACHE:
        _CACHE[flags] = _build_program(flags)
    nc = _CACHE[flags]

    trace = bool(_RUN_OPTS.get("trace"))
    res = run_bass_kernel_spmd(
        nc, in_maps, core_ids=list(range(NCORES)), trace=trace,
        **_RUN_OPTS.get("kwargs", {}),
    )
    _RUN_OPTS["last_result"] = res

    # device rows are step-major per core; reorder to token-major
    slabs = [
        res.results[ci]["scores"].reshape(F, TC, V).transpose(1, 0, 2)
        for ci in range(NCORES)
    ]
    out = np.concatenate(slabs, axis=0)  # [T, F, V]
    out = _eos_fixup(out, gold, c0)
    return np.ascontiguousarray(out.reshape(1, T * F, V))


# knobs used by test.py (harness just calls kernel())
_RUN_OPTS = {"trace": False, "kwargs": {}}


# revision 35
# speedup vs baseline: 1.0123x; 1.0123x over previous
"""Trainium2 Bass kernel for nn_Model_34316788695805 (ragged_sequence).

Model: per-token char-level encoder GRU (C=8 steps) -> decoder GRU
(F=32 steps, teacher forced) -> vocab projection scores.

Sharding: token-parallel over 8 NeuronCores (32 tokens/core).  Each core
runs the full enc+dec GRU for its tokens and the full vocab projection,
producing a contiguous [1024, 10000] slab of the output.  No collectives;
the host concatenates the slabs.

Device layout: hidden dim H=1024 lives on partitions (8 chunks of 128),
tokens on the free axis.  GRU gate matmuls keep W_hh^T stationary
(bf16, FWL) and stream h (bf16).  Gate arithmetic is fp32 on DVE/ACT.
The vocab projection is a single batched matmul at the end over all
32 steps (lhsT = transposed hidden states, rhs = streamed out_W^T slabs).

Host-side work is limited to sharding/layout prep: embedding gathers,
mean-pooling of h0, weight transposes/casts, and EOS-freeze fixup
(a no-op for the generated inputs, which contain no EOS).
"""

import numpy as np
import ml_dtypes
from contextlib import ExitStack

import concourse.bass as bass
import concourse.mybir as mybir
import concourse.tile as tile
from concourse import bacc
from concourse.bass_utils import run_bass_kernel_spmd

# Problem constants (hardcoded per spec)
T, F, C, V, H, E, S = 256, 32, 8, 10000, 1024, 256, 512
PAD, BOS, EOS = 0, 1, 2
NCORES = 8
TC = T // NCORES          # 32 tokens per core
TS = TC * F               # 1024 (token,step) pairs per core
KH = H // 128             # 8 k-chunks of hidden
KE = E // 128             # 2 k-chunks of embedding
MG = H // 128             # 8 m-chunks per gate
VCH = 512                 # vocab chunk (one PSUM bank of fp32)
NV = (V + VCH - 1) // VCH  # 20 chunks
VPAD = NV * VCH           # 10240

F32 = mybir.dt.float32
BF16 = mybir.dt.bfloat16
AF = mybir.ActivationFunctionType
npbf16 = ml_dtypes.bfloat16

_CACHE = {}


def _to_lhsT_layout(w):
    """[M, K] weight -> [128, K//128, M] array so that
    arr[p, k, m] = w[m, 128*k + p]; lhsT tile (k, m0) = arr[:, k, m0:m0+128]."""
    M, K = w.shape
    return np.ascontiguousarray(w.T.reshape(K // 128, 128, M).transpose(1, 0, 2))


def _cols_layout(x):
    """[N, K] -> [128, K//128, N]: arr[p, k, n] = x[n, 128*k + p] (rhs/moving)."""
    N, K = x.shape
    return np.ascontiguousarray(x.T.reshape(K // 128, 128, N).transpose(1, 0, 2))


def _build_program(flags):
    """Build + compile the Bacc/Tile program. flags: (gib_enc, ghn_enc,
    gib_dec, ghn_dec, outb) nonzero-bias booleans."""
    has_gib_enc, has_ghn_enc, has_gib_dec, has_ghn_dec, has_outb = flags

    nc = bacc.Bacc(
        "TRN2",
        target_bir_lowering=False,
        debug=False,
        enable_asserts=False,
        num_devices=NCORES,
    )

    # ---- DRAM I/O ----
    d_h0 = nc.dram_tensor("h0T", [128, KH, TC], F32, kind="ExternalInput").ap()
    d_xenc = nc.dram_tensor("xencT", [128, KE, C * TC], BF16, kind="ExternalInput").ap()
    d_xdec = nc.dram_tensor("xdecT", [128, KE, TS], BF16, kind="ExternalInput").ap()
    d_whh_e = nc.dram_tensor("whhTe", [128, KH, 3 * H], BF16, kind="ExternalInput").ap()
    d_whh_d = nc.dram_tensor("whhTd", [128, KH, 3 * H], BF16, kind="ExternalInput").ap()
    d_wih_e = nc.dram_tensor("wihTe", [128, KE, 3 * H], BF16, kind="ExternalInput").ap()
    d_wih_d = nc.dram_tensor("wihTd", [128, KE, 3 * H], BF16, kind="ExternalInput").ap()
    d_ow = nc.dram_tensor("owT", [NV, 128, KH, VCH], BF16, kind="ExternalInput").ap()
    d_gib_e = d_gib_d = d_ghn_e = d_ghn_d = d_outb = None
    if has_gib_enc:
        d_gib_e = nc.dram_tensor("gibE", [128, 24], F32, kind="ExternalInput").ap()
    if has_gib_dec:
        d_gib_d = nc.dram_tensor("gibD", [128, 24], F32, kind="ExternalInput").ap()
    if has_ghn_enc:
        d_ghn_e = nc.dram_tensor("ghnE", [128, MG], F32, kind="ExternalInput").ap()
    if has_ghn_dec:
        d_ghn_d = nc.dram_tensor("ghnD", [128, MG], F32, kind="ExternalInput").ap()
    if has_outb:
        d_outb = nc.dram_tensor("outb", [1, VPAD], BF16, kind="ExternalInput").ap()
    d_scores = nc.dram_tensor("scores", [TS, V], F32, kind="ExternalOutput").ap()

    with tile.TileContext(nc) as tc, ExitStack() as ctx:
        wpool = ctx.enter_context(tc.tile_pool(name="weights", bufs=1))
        whpool = ctx.enter_context(tc.tile_pool(name="whh", bufs=2))
        wipool = ctx.enter_context(tc.tile_pool(name="wih", bufs=1))
        gipool = ctx.enter_context(tc.tile_pool(name="gi", bufs=2))
        hpool = ctx.enter_context(tc.tile_pool(name="h", bufs=2))
        gpool = ctx.enter_context(tc.tile_pool(name="gates", bufs=2))
        spool = ctx.enter_context(tc.tile_pool(name="slab", bufs=3))
        stpool = ctx.enter_context(tc.tile_pool(name="staging", bufs=4))
        ps_gh = ctx.enter_context(tc.tile_pool(name="ps_gh", bufs=1, space="PSUM"))
        ps_gi = ctx.enter_context(tc.tile_pool(name="ps_gi", bufs=2, space="PSUM"))
        ps_sc = ctx.enter_context(tc.tile_pool(name="ps_sc", bufs=3, space="PSUM"))

        # ---- resident weights / inputs (small/urgent first: one FIFO ring) ----
        xenc = wpool.tile([128, KE, C * TC], BF16, tag="xenc")
        nc.sync.dma_start(xenc[:], d_xenc)
        h_f = hpool.tile([128, KH, TC], F32, tag="hf")
        nc.sync.dma_start(h_f[:], d_h0)
        h_b = hpool.tile([128, KH, TC], BF16, tag="hb")
        nc.vector.tensor_copy(h_b[:], h_f[:])
        wih_e = wipool.tile([128, KE, 3 * H], BF16, tag="wih")
        nc.sync.dma_start(wih_e[:], d_wih_e)
        # gate-split DMAs: enc step 0's r-gate matmuls start after 1/3 loads
        whh_e = whpool.tile([128, KH, 3 * H], BF16, tag="whh")
        for g in (0, 2, 1):
            nc.sync.dma_start(
                whh_e[:, :, g * H : (g + 1) * H], d_whh_e[:, :, g * H : (g + 1) * H]
            )
        xdec = wpool.tile([128, KE, TS], BF16, tag="xdec")
        nc.sync.dma_start(xdec[:], d_xdec)
        # hidden-state history (step-major columns: ts = s*TC + t), bf16;
        # the output DMA access pattern restores token-major row order
        hstT = wpool.tile([128, KH, F, TC], BF16, tag="hstT")

        gib_e = gib_d = ghn_e = ghn_d = None
        if has_gib_enc:
            gib_e = wpool.tile([128, 24], F32, tag="gib_e")
            nc.sync.dma_start(gib_e[:], d_gib_e)
        if has_gib_dec:
            gib_d = wpool.tile([128, 24], F32, tag="gib_d")
            nc.sync.dma_start(gib_d[:], d_gib_d)
        if has_ghn_enc:
            ghn_e = wpool.tile([128, MG], F32, tag="ghn_e")
            nc.sync.dma_start(ghn_e[:], d_ghn_e)
        if has_ghn_dec:
            ghn_d = wpool.tile([128, MG], F32, tag="ghn_d")
            nc.sync.dma_start(ghn_d[:], d_ghn_d)
        ones_row = None
        if has_outb:
            ones_row = wpool.tile([1, 128], BF16, tag="ones")
            nc.vector.memset(ones_row[:], 1.0)
        outb_sb = None
        if has_outb:
            outb_sb = wpool.tile([1, VPAD], BF16, tag="outb")
            nc.sync.dma_start(outb_sb[:], d_outb)

        def gi_batch(gi_tile, wih, x_ap, nsteps, gib):
            """gi[gate, mchunk, ts] = W_ih @ x (+ gate bias) for a block of
            steps. x_ap: [128, KE, nsteps*TC] bf16."""
            n = nsteps * TC
            for g in range(3):
                for j in range(MG):
                    m = g * H + j * 128
                    ps = ps_gi.tile([128, n], F32, tag="ps_gi")
                    for k in range(KE):
                        nc.tensor.matmul(
                            ps[:, :n],
                            wih[:, k, m : m + 128],
                            x_ap[:, k, :],
                            start=(k == 0),
                            stop=(k == KE - 1),
                        )
                    if gib is not None:
                        nc.scalar.activation(
                            gi_tile[:, g, j, :n], ps[:, :n], AF.Identity,
                            bias=gib[:, g * MG + j : g * MG + j + 1],
                        )
                    else:
                        nc.scalar.copy(gi_tile[:, g, j, :n], ps[:, :n])

        def gru_step(whh, gi_tile, s_in_chunk, ghn, hst_write_step=None):
            """One GRU step: h (h_f/h_b tiles, closed over) -> new h.
            gi_tile slice cols [s_in_chunk*TC, +TC]."""
            nonlocal h_f, h_b
            c0 = s_in_chunk * TC
            gh_r = ps_gh.tile([128, MG, TC], F32, tag="gh_r")
            gh_z = ps_gh.tile([128, MG, TC], F32, tag="gh_z")
            gh_n = ps_gh.tile([128, MG, TC], F32, tag="gh_n")
            # emission order r, n, z: the r/n gate chains overlap the
            # remaining matmuls; only the short z chain trails the step.
            for g, ps in ((0, gh_r), (2, gh_n), (1, gh_z)):
                for j in range(MG):
                    m = g * H + j * 128
                    for k in range(KH):
                        nc.tensor.matmul(
                            ps[:, j, :],
                            whh[:, k, m : m + 128],
                            h_b[:, k, :],
                            start=(k == 0),
                            stop=(k == KH - 1),
                        )
            gi_r = gi_tile[:, 0, :, c0 : c0 + TC]
            gi_z = gi_tile[:, 1, :, c0 : c0 + TC]
            gi_n = gi_tile[:, 2, :, c0 : c0 + TC]

            r_pre = gpool.tile([128, MG, TC], F32, tag="r_pre")
            nc.vector.tensor_add(r_pre[:], gi_r, gh_r[:])
            r = gpool.tile([128, MG, TC], F32, tag="r")
            nc.scalar.activation(r[:], r_pre[:], AF.Sigmoid)
            # n path
            if ghn is not None:
                ghn_sb = gpool.tile([128, MG, TC], F32, tag="ghn_sb")
                for j in range(MG):
                    nc.scalar.activation(
                        ghn_sb[:, j, :], gh_n[:, j, :], AF.Identity,
                        bias=ghn[:, j : j + 1],
                    )
                n_src = ghn_sb[:]
            else:
                n_src = gh_n[:]
            rn = gpool.tile([128, MG, TC], F32, tag="rn")
            nc.vector.tensor_mul(rn[:], r[:], n_src)
            n_pre = gpool.tile([128, MG, TC], F32, tag="n_pre")
            nc.vector.tensor_add(n_pre[:], rn[:], gi_n)
            n = gpool.tile([128, MG, TC], F32, tag="n")
            nc.scalar.activation(n[:], n_pre[:], AF.Tanh)
            # d = h - n (fp32 h keeps recurrence precision; off critical path)
            d = gpool.tile([128, MG, TC], F32, tag="d")
            nc.vector.tensor_sub(d[:], h_f[:], n[:])
            # z tail: z_pre -> sigmoid -> zd -> h (bf16 direct for the PE)
            z_pre = gpool.tile([128, MG, TC], F32, tag="z_pre")
            nc.vector.tensor_add(z_pre[:], gi_z, gh_z[:])
            z = gpool.tile([128, MG, TC], F32, tag="z")
            nc.scalar.activation(z[:], z_pre[:], AF.Sigmoid)
            zd = gpool.tile([128, MG, TC], F32, tag="zd")
            nc.vector.tensor_mul(zd[:], z[:], d[:])
            h_b = hpool.tile([128, KH, TC], BF16, tag="hb")
            nc.vector.tensor_add(h_b[:], n[:], zd[:])
            h_f = hpool.tile([128, KH, TC], F32, tag="hf")
            nc.vector.tensor_add(h_f[:], n[:], zd[:])
            if hst_write_step is not None:
                nc.scalar.copy(hstT[:, :, hst_write_step, :], h_b[:])

        # ---- encoder ----
        GCH = 4  # steps per gi chunk
        gi_t = gipool.tile([128, 3, MG, GCH * TC], F32, tag="gi")
        gi_batch(gi_t, wih_e, xenc[:, :, 0 : GCH * TC], GCH, gib_e)
        for s in range(C):
            g = s // GCH
            gru_step(whh_e, gi_t, s % GCH, ghn_e)
            if s == 1:
                # dec weights load during the encoder (DMA is idle here)
                wih_d = wipool.tile([128, KE, 3 * H], BF16, tag="wih")
                nc.sync.dma_start(wih_d[:], d_wih_d)
                whh_d = whpool.tile([128, KH, 3 * H], BF16, tag="whh")
                nc.sync.dma_start(whh_d[:], d_whh_d)
            if s % GCH == 0 and g + 1 < C // GCH:
                gi_next = gipool.tile([128, 3, MG, GCH * TC], F32, tag="gi")
                gi_batch(
                    gi_next, wih_e,
                    xenc[:, :, (g + 1) * GCH * TC : (g + 2) * GCH * TC],
                    GCH, gib_e,
                )
            if s % GCH == GCH - 1 and g + 1 < C // GCH:
                gi_t = gi_next

        # output rows are stored step-major (row = s*TC + t, contiguous
        # per block); the host reorders rows to token-major afterwards
        def scores_block(sb, c, slab, par):
            """Scores for step block sb (4 steps) x vocab chunk c."""
            ncols = min(VCH, V - c * VCH)
            ps = ps_sc.tile([128, VCH], F32, tag="ps_sc")
            for k in range(KH):
                nc.tensor.matmul(
                    ps[:],
                    hstT[:, k, 4 * sb : 4 * sb + 4, :],
                    slab[:, k, :],
                    start=(k == 0),
                    stop=False if has_outb else (k == KH - 1),
                )
            if has_outb:
                nc.tensor.matmul(
                    ps[:], ones_row[:], outb_sb[:, c * VCH : (c + 1) * VCH],
                    start=False, stop=True,
                )
            st = stpool.tile([128, VCH], F32, tag="st")
            if par % 2 == 0:
                nc.vector.tensor_copy(st[:], ps[:])
            else:
                nc.scalar.copy(st[:], ps[:])
            # stores on the ACT HWDGE ring; slab loads stay on SP's
            nc.scalar.dma_start(
                d_scores[128 * sb : 128 * (sb + 1), c * VCH : c * VCH + ncols],
                st[:, :ncols],
            )

        # ---- decoder (scores for the first vocab chunks fill step tails) ----
        from collections import deque

        N_INTER = 3
        inter_slabs = []
        for c in range(N_INTER):
            slab = spool.tile([128, KH, VCH], BF16, tag="slab")
            nc.sync.dma_start(slab[:], d_ow[c])
            inter_slabs.append(slab)

        pending = deque()
        gi_t = gipool.tile([128, 3, MG, GCH * TC], F32, tag="gi")
        gi_batch(gi_t, wih_d, xdec[:, :, 0 : GCH * TC], GCH, gib_d)
        for s in range(F):
            g = s // GCH
            gru_step(whh_d, gi_t, s % GCH, ghn_d, hst_write_step=s)
            if s % GCH == 0 and g + 1 < F // GCH:
                gi_next = gipool.tile([128, 3, MG, GCH * TC], F32, tag="gi")
                gi_batch(
                    gi_next, wih_d,
                    xdec[:, :, (g + 1) * GCH * TC : (g + 2) * GCH * TC],
                    GCH, gib_d,
                )
            if s % GCH == GCH - 1 and g + 1 < F // GCH:
                gi_t = gi_next
            # one scores block from an ALREADY-COMPLETE step block per tail
            if pending:
                sb, c = pending.popleft()
                scores_block(sb, c, inter_slabs[c], sb + c)
            if s % 4 == 3:
                pending.extend((s // 4, c) for c in range(N_INTER))
        for sb, c in pending:
            scores_block(sb, c, inter_slabs[c], sb + c)

        # ---- remaining vocab projection ----
        for c in range(N_INTER, NV):
            slab = spool.tile([128, KH, VCH], BF16, tag="slab")
            nc.sync.dma_start(slab[:], d_ow[c])
            for sb in range(F // 4):
                scores_block(sb, c, slab, sb + c)

    nc.compile()
    return nc


def _prep_inputs(token_ctx, char_emb_w, enc_W_ih, enc_W_hh, enc_b_ih, enc_b_hh,
                 dec_W_ih, dec_W_hh, dec_b_ih, dec_b_hh, out_W, out_b,
                 in_sent_token_chars, out_chars):
    """Host-side sharding/layout prep. Returns (in_maps, flags, fixup_info)."""
    tcarr = np.asarray(in_sent_token_chars)[0].reshape(T, C, 3)
    chars = tcarr[:, :, 2]
    xt = tcarr[:, :, 1]
    token_ctx = np.asarray(token_ctx)[0]          # [S, H]
    char_emb_w = np.asarray(char_emb_w)           # [V, E]
    out_chars = np.asarray(out_chars)[0]          # [1 + T*F]

    h0 = token_ctx[xt].mean(axis=1).astype(np.float32)      # [T, H]
    x_enc = char_emb_w[chars]                                # [T, C, E]
    gold = out_chars[1 : 1 + T * F].reshape(T, F)
    c0 = out_chars[0]
    c_in = np.concatenate(
        [np.full((T, 1), c0, dtype=gold.dtype), gold[:, :-1]], axis=1
    )                                                        # [T, F]
    x_dec = char_emb_w[c_in]                                 # [T, F, E]

    # shared (replicated) weight layouts
    whhTe = _to_lhsT_layout(np.asarray(enc_W_hh)).astype(npbf16)
    whhTd = _to_lhsT_layout(np.asarray(dec_W_hh)).astype(npbf16)
    wihTe = _to_lhsT_layout(np.asarray(enc_W_ih)).astype(npbf16)
    wihTd = _to_lhsT_layout(np.asarray(dec_W_ih)).astype(npbf16)
    owpad = np.zeros((VPAD, H), np.float32)
    owpad[:V] = np.asarray(out_W)
    owT = np.ascontiguousarray(
        owpad.reshape(NV, VCH, KH, 128).transpose(0, 3, 2, 1)
    ).astype(npbf16)                                          # [NV,128,KH,VCH]

    def gate_bias(b_ih, b_hh):
        b_ih = np.asarray(b_ih); b_hh = np.asarray(b_hh)
        gib = np.concatenate([b_ih[: 2 * H] + b_hh[: 2 * H], b_ih[2 * H :]])
        ghn = b_hh[2 * H :]
        gib_l = np.ascontiguousarray(gib.reshape(24, 128).T).astype(np.float32)
        ghn_l = np.ascontiguousarray(ghn.reshape(MG, 128).T).astype(np.float32)
        return gib_l, ghn_l, bool(np.any(gib)), bool(np.any(ghn))

    gibE, ghnE, has_gib_e, has_ghn_e = gate_bias(enc_b_ih, enc_b_hh)
    gibD, ghnD, has_gib_d, has_ghn_d = gate_bias(dec_b_ih, dec_b_hh)
    out_b = np.asarray(out_b)
    has_outb = bool(np.any(out_b))
    outb_pad = np.zeros((1, VPAD), npbf16)
    outb_pad[0, :V] = out_b.astype(npbf16)

    flags = (has_gib_e, has_ghn_e, has_gib_d, has_ghn_d, has_outb)

    in_maps = []
    for ci in range(NCORES):
        sl = slice(ci * TC, (ci + 1) * TC)
        h0T = np.ascontiguousarray(
            h0[sl].T.reshape(KH, 128, TC).transpose(1, 0, 2)
        )
        # enc ts = c*TC + t (step-major)
        xe = x_enc[sl].transpose(1, 0, 2).reshape(C * TC, E).astype(np.float32)
        xencT = _cols_layout(xe).astype(npbf16)
        # dec ts = s*TC + t (step-major)
        xd = x_dec[sl].transpose(1, 0, 2).reshape(TS, E).astype(np.float32)
        xdecT = _cols_layout(xd).astype(npbf16)
        m = {
            "h0T": h0T, "xencT": xencT, "xdecT": xdecT,
            "whhTe": whhTe, "whhTd": whhTd, "wihTe": wihTe, "wihTd": wihTd,
            "owT": owT,
        }
        if has_gib_e: m["gibE"] = gibE
        if has_gib_d: m["gibD"] = gibD
        if has_ghn_e: m["ghnE"] = ghnE
        if has_ghn_d: m["ghnD"] = ghnD
        if has_outb: m["outb"] = outb_pad
        in_maps.append(m)

    return in_maps, flags, (gold, c0)


def _eos_fixup(scores, gold, c0):
    """Apply the reference's EOS freeze/pad semantics on the host.
    scores: [T, F, V] (modified in place)."""
    if c0 != EOS and not np.any(gold == EOS):
        return scores
    done0 = c0 == EOS
    for t in range(T):
        hits = np.nonzero(gold[t] == EOS)[0]
        if done0:
            first_done = 0
        elif len(hits):
            first_done = int(hits[0]) + 1
        else:
            continue
        if first_done == 0:
            scores[t, :, :] = 0.0
        elif first_done < F:
            scores[t, first_done:, :] = scores[t, first_done - 1, :]
    return scores


def kernel(**inputs) -> np.ndarray:
    assert int(inputs["max_tokens"]) == T
    assert int(inputs["max_form_len"]) == F
    assert int(inputs["use_teacher_forcing"]) == 1

    in_maps, flags, (gold, c0) = _prep_inputs(
        inputs["token_ctx"], inputs["char_emb_w"],
        inputs["enc_W_ih"], inputs["enc_W_hh"], inputs["enc_b_ih"], inputs["enc_b_hh"],
        inputs["dec_W_ih"], inputs["dec_W_hh"], inputs["dec_b_ih"], inputs["dec_b_hh"],
        inputs["out_W"], inputs["out_b"],
        inputs["in_sent_token_chars"], inputs["out_chars"],
    )

    if flags not in _CACHE:
        _CACHE[flags] = _build_program(flags)
    nc = _CACHE[flags]

    trace = bool(_RUN_OPTS.get("trace"))
    res = run_bass_kernel_spmd(
        nc, in_maps, core_ids=list(range(NCORES)), trace=trace,
        **_RUN_OPTS.get("kwargs", {}),
    )
    _RUN_OPTS["last_result"] = res

    # device rows are step-major per core; reorder to token-major
    slabs = [
        res.results[ci]["scores"].reshape(F, TC, V).transpose(1, 0, 2)
        for ci in range(NCORES)
    ]
    out = np.concatenate(slabs, axis=0)  # [T, F, V]
    out = _eos_fixup(out, gold, c0)
    return np.ascontiguousarray(out.reshape(1, T * F, V))


# knobs used by test.py (harness just calls kernel())
_RUN_OPTS = {"trace": False, "kwargs": {}}


# revision 36
# speedup vs baseline: 1.0277x; 1.0152x over previous
"""Trainium2 Bass kernel for nn_Model_34316788695805 (ragged_sequence).

Model: per-token char-level encoder GRU (C=8 steps) -> decoder GRU
(F=32 steps, teacher forced) -> vocab projection scores.

Sharding: token-parallel over 8 NeuronCores (32 tokens/core).  Each core
runs the full enc+dec GRU for its tokens and the full vocab projection,
producing a contiguous [1024, 10000] slab of the output.  No collectives;
the host concatenates the slabs.

Device layout: hidden dim H=1024 lives on partitions (8 chunks of 128),
tokens on the free axis.  GRU gate matmuls keep W_hh^T stationary
(bf16, FWL) and stream h (bf16).  Gate arithmetic is fp32 on DVE/ACT.
The vocab projection is a single batched matmul at the end over all
32 steps (lhsT = transposed hidden states, rhs = streamed out_W^T slabs).

Host-side work is limited to sharding/layout prep: embedding gathers,
mean-pooling of h0, weight transposes/casts, and EOS-freeze fixup
(a no-op for the generated inputs, which contain no EOS).
"""

import numpy as np
import ml_dtypes
from contextlib import ExitStack

import concourse.bass as bass
import concourse.mybir as mybir
import concourse.tile as tile
from concourse import bacc
from concourse.bass_utils import run_bass_kernel_spmd

# Problem constants (hardcoded per spec)
T, F, C, V, H, E, S = 256, 32, 8, 10000, 1024, 256, 512
PAD, BOS, EOS = 0, 1, 2
NCORES = 8
TC = T // NCORES          # 32 tokens per core
TS = TC * F               # 1024 (token,step) pairs per core
KH = H // 128             # 8 k-chunks of hidden
KE = E // 128             # 2 k-chunks of embedding
MG = H // 128             # 8 m-chunks per gate
VCH = 512                 # vocab chunk (one PSUM bank of fp32)
NV = (V + VCH - 1) // VCH  # 20 chunks
VPAD = NV * VCH           # 10240

F32 = mybir.dt.float32
BF16 = mybir.dt.bfloat16
AF = mybir.ActivationFunctionType
npbf16 = ml_dtypes.bfloat16

_CACHE = {}


def _to_lhsT_layout(w):
    """[M, K] weight -> [128, K//128, M] array so that
    arr[p, k, m] = w[m, 128*k + p]; lhsT tile (k, m0) = arr[:, k, m0:m0+128]."""
    M, K = w.shape
    return np.ascontiguousarray(w.T.reshape(K // 128, 128, M).transpose(1, 0, 2))


def _cols_layout(x):
    """[N, K] -> [128, K//128, N]: arr[p, k, n] = x[n, 128*k + p] (rhs/moving)."""
    N, K = x.shape
    return np.ascontiguousarray(x.T.reshape(K // 128, 128, N).transpose(1, 0, 2))


def _build_program(flags):
    """Build + compile the Bacc/Tile program. flags: (gib_enc, ghn_enc,
    gib_dec, ghn_dec, outb) nonzero-bias booleans."""
    has_gib_enc, has_ghn_enc, has_gib_dec, has_ghn_dec, has_outb = flags

    nc = bacc.Bacc(
        "TRN2",
        target_bir_lowering=False,
        debug=False,
        enable_asserts=False,
        num_devices=NCORES,
    )

    # ---- DRAM I/O ----
    d_h0 = nc.dram_tensor("h0T", [128, KH, TC], F32, kind="ExternalInput").ap()
    d_xenc = nc.dram_tensor("xencT", [128, KE, C * TC], BF16, kind="ExternalInput").ap()
    d_xdec = nc.dram_tensor("xdecT", [128, KE, TS], BF16, kind="ExternalInput").ap()
    d_whh_e = nc.dram_tensor("whhTe", [128, KH, 3 * H], BF16, kind="ExternalInput").ap()
    d_whh_d = nc.dram_tensor("whhTd", [128, KH, 3 * H], BF16, kind="ExternalInput").ap()
    d_wih_e = nc.dram_tensor("wihTe", [128, KE, 3 * H], BF16, kind="ExternalInput").ap()
    d_wih_d = nc.dram_tensor("wihTd", [128, KE, 3 * H], BF16, kind="ExternalInput").ap()
    d_ow = nc.dram_tensor("owT", [NV, 128, KH, VCH], BF16, kind="ExternalInput").ap()
    d_gib_e = d_gib_d = d_ghn_e = d_ghn_d = d_outb = None
    if has_gib_enc:
        d_gib_e = nc.dram_tensor("gibE", [128, 24], F32, kind="ExternalInput").ap()
    if has_gib_dec:
        d_gib_d = nc.dram_tensor("gibD", [128, 24], F32, kind="ExternalInput").ap()
    if has_ghn_enc:
        d_ghn_e = nc.dram_tensor("ghnE", [128, MG], F32, kind="ExternalInput").ap()
    if has_ghn_dec:
        d_ghn_d = nc.dram_tensor("ghnD", [128, MG], F32, kind="ExternalInput").ap()
    if has_outb:
        d_outb = nc.dram_tensor("outb", [1, VPAD], BF16, kind="ExternalInput").ap()
    d_scores = nc.dram_tensor("scores", [TS, V], F32, kind="ExternalOutput").ap()

    with tile.TileContext(nc) as tc, ExitStack() as ctx:
        wpool = ctx.enter_context(tc.tile_pool(name="weights", bufs=1))
        whpool = ctx.enter_context(tc.tile_pool(name="whh", bufs=2))
        wipool = ctx.enter_context(tc.tile_pool(name="wih", bufs=1))
        gipool = ctx.enter_context(tc.tile_pool(name="gi", bufs=2))
        hpool = ctx.enter_context(tc.tile_pool(name="h", bufs=2))
        gpool = ctx.enter_context(tc.tile_pool(name="gates", bufs=2))
        spool = ctx.enter_context(tc.tile_pool(name="slab", bufs=3))
        stpool = ctx.enter_context(tc.tile_pool(name="staging", bufs=4))
        ps_gh = ctx.enter_context(tc.tile_pool(name="ps_gh", bufs=1, space="PSUM"))
        ps_gi = ctx.enter_context(tc.tile_pool(name="ps_gi", bufs=2, space="PSUM"))
        ps_sc = ctx.enter_context(tc.tile_pool(name="ps_sc", bufs=3, space="PSUM"))

        # ---- resident weights / inputs (small/urgent first: one FIFO ring) ----
        xenc = wpool.tile([128, KE, C * TC], BF16, tag="xenc")
        nc.sync.dma_start(xenc[:], d_xenc)
        h_f = hpool.tile([128, KH, TC], F32, tag="hf")
        nc.sync.dma_start(h_f[:], d_h0)
        h_b = hpool.tile([128, KH, TC], BF16, tag="hb")
        nc.vector.tensor_copy(h_b[:], h_f[:])
        wih_e = wipool.tile([128, KE, 3 * H], BF16, tag="wih")
        nc.sync.dma_start(wih_e[:], d_wih_e)
        # gate-split DMAs: enc step 0's r-gate matmuls start after 1/3 loads
        whh_e = whpool.tile([128, KH, 3 * H], BF16, tag="whh")
        for g in (0, 2, 1):
            nc.sync.dma_start(
                whh_e[:, :, g * H : (g + 1) * H], d_whh_e[:, :, g * H : (g + 1) * H]
            )
        xdec = wpool.tile([128, KE, TS], BF16, tag="xdec")
        nc.sync.dma_start(xdec[:], d_xdec)
        # hidden-state history (step-major columns: ts = s*TC + t), bf16;
        # the output DMA access pattern restores token-major row order
        hstT = wpool.tile([128, KH, F, TC], BF16, tag="hstT")

        gib_e = gib_d = ghn_e = ghn_d = None
        if has_gib_enc:
            gib_e = wpool.tile([128, 24], F32, tag="gib_e")
            nc.sync.dma_start(gib_e[:], d_gib_e)
        if has_gib_dec:
            gib_d = wpool.tile([128, 24], F32, tag="gib_d")
            nc.sync.dma_start(gib_d[:], d_gib_d)
        if has_ghn_enc:
            ghn_e = wpool.tile([128, MG], F32, tag="ghn_e")
            nc.sync.dma_start(ghn_e[:], d_ghn_e)
        if has_ghn_dec:
            ghn_d = wpool.tile([128, MG], F32, tag="ghn_d")
            nc.sync.dma_start(ghn_d[:], d_ghn_d)
        ones_row = None
        if has_outb:
            ones_row = wpool.tile([1, 128], BF16, tag="ones")
            nc.vector.memset(ones_row[:], 1.0)
        outb_sb = None
        if has_outb:
            outb_sb = wpool.tile([1, VPAD], BF16, tag="outb")
            nc.sync.dma_start(outb_sb[:], d_outb)

        def gi_batch(gi_tile, wih, x_ap, nsteps, gib):
            """gi[gate, mchunk, ts] = W_ih @ x (+ gate bias) for a block of
            steps. x_ap: [128, KE, nsteps*TC] bf16."""
            n = nsteps * TC
            for g in range(3):
                for j in range(MG):
                    m = g * H + j * 128
                    ps = ps_gi.tile([128, n], F32, tag="ps_gi")
                    for k in range(KE):
                        nc.tensor.matmul(
                            ps[:, :n],
                            wih[:, k, m : m + 128],
                            x_ap[:, k, :],
                            start=(k == 0),
                            stop=(k == KE - 1),
                        )
                    if gib is not None:
                        nc.scalar.activation(
                            gi_tile[:, g, j, :n], ps[:, :n], AF.Identity,
                            bias=gib[:, g * MG + j : g * MG + j + 1],
                        )
                    else:
                        nc.scalar.copy(gi_tile[:, g, j, :n], ps[:, :n])

        def gru_step(whh, gi_tile, s_in_chunk, ghn, hst_write_step=None):
            """One GRU step: h (h_f/h_b tiles, closed over) -> new h.
            gi_tile slice cols [s_in_chunk*TC, +TC]."""
            nonlocal h_f, h_b
            c0 = s_in_chunk * TC
            gh_r = ps_gh.tile([128, MG, TC], F32, tag="gh_r")
            gh_z = ps_gh.tile([128, MG, TC], F32, tag="gh_z")
            gh_n = ps_gh.tile([128, MG, TC], F32, tag="gh_n")
            # emission order r, n, z: the r/n gate chains overlap the
            # remaining matmuls; only the short z chain trails the step.
            for g, ps in ((0, gh_r), (2, gh_n), (1, gh_z)):
                for j in range(MG):
                    m = g * H + j * 128
                    for k in range(KH):
                        nc.tensor.matmul(
                            ps[:, j, :],
                            whh[:, k, m : m + 128],
                            h_b[:, k, :],
                            start=(k == 0),
                            stop=(k == KH - 1),
                        )
            gi_r = gi_tile[:, 0, :, c0 : c0 + TC]
            gi_z = gi_tile[:, 1, :, c0 : c0 + TC]
            gi_n = gi_tile[:, 2, :, c0 : c0 + TC]

            r_pre = gpool.tile([128, MG, TC], F32, tag="r_pre")
            nc.vector.tensor_add(r_pre[:], gi_r, gh_r[:])
            r = gpool.tile([128, MG, TC], F32, tag="r")
            nc.scalar.activation(r[:], r_pre[:], AF.Sigmoid)
            # n path
            if ghn is not None:
                ghn_sb = gpool.tile([128, MG, TC], F32, tag="ghn_sb")
                for j in range(MG):
                    nc.scalar.activation(
                        ghn_sb[:, j, :], gh_n[:, j, :], AF.Identity,
                        bias=ghn[:, j : j + 1],
                    )
                n_src = ghn_sb[:]
            else:
                n_src = gh_n[:]
            rn = gpool.tile([128, MG, TC], F32, tag="rn")
            nc.vector.tensor_mul(rn[:], r[:], n_src)
            n_pre = gpool.tile([128, MG, TC], F32, tag="n_pre")
            nc.vector.tensor_add(n_pre[:], rn[:], gi_n)
            n = gpool.tile([128, MG, TC], F32, tag="n")
            nc.scalar.activation(n[:], n_pre[:], AF.Tanh)
            # d = h - n (fp32 h keeps recurrence precision; off critical path)
            d = gpool.tile([128, MG, TC], F32, tag="d")
            nc.vector.tensor_sub(d[:], h_f[:], n[:])
            # z tail: z_pre -> sigmoid -> zd -> h (bf16 direct for the PE)
            z_pre = gpool.tile([128, MG, TC], F32, tag="z_pre")
            nc.vector.tensor_add(z_pre[:], gi_z, gh_z[:])
            z = gpool.tile([128, MG, TC], F32, tag="z")
            nc.scalar.activation(z[:], z_pre[:], AF.Sigmoid)
            zd = gpool.tile([128, MG, TC], F32, tag="zd")
            nc.vector.tensor_mul(zd[:], z[:], d[:])
            h_b = hpool.tile([128, KH, TC], BF16, tag="hb")
            nc.vector.tensor_add(h_b[:], n[:], zd[:])
            h_f = hpool.tile([128, KH, TC], F32, tag="hf")
            nc.vector.tensor_add(h_f[:], n[:], zd[:])
            if hst_write_step is not None:
                nc.scalar.copy(hstT[:, :, hst_write_step, :], h_b[:])

        # ---- encoder ----
        GCH = 4  # steps per gi chunk
        gi_t = gipool.tile([128, 3, MG, GCH * TC], F32, tag="gi")
        gi_batch(gi_t, wih_e, xenc[:, :, 0 : GCH * TC], GCH, gib_e)
        for s in range(C):
            g = s // GCH
            gru_step(whh_e, gi_t, s % GCH, ghn_e)
            if s == 1:
                # dec weights load during the encoder (DMA is idle here)
                wih_d = wipool.tile([128, KE, 3 * H], BF16, tag="wih")
                nc.sync.dma_start(wih_d[:], d_wih_d)
                whh_d = whpool.tile([128, KH, 3 * H], BF16, tag="whh")
                nc.sync.dma_start(whh_d[:], d_whh_d)
            if s % GCH == 0 and g + 1 < C // GCH:
                gi_next = gipool.tile([128, 3, MG, GCH * TC], F32, tag="gi")
                gi_batch(
                    gi_next, wih_e,
                    xenc[:, :, (g + 1) * GCH * TC : (g + 2) * GCH * TC],
                    GCH, gib_e,
                )
            if s % GCH == GCH - 1 and g + 1 < C // GCH:
                gi_t = gi_next

        # output rows are stored step-major (row = s*TC + t, contiguous
        # per block); the host reorders rows to token-major afterwards
        def scores_block(sb, c, slab, par):
            """Scores for step block sb (4 steps) x vocab chunk c."""
            ncols = min(VCH, V - c * VCH)
            ps = ps_sc.tile([128, VCH], F32, tag="ps_sc")
            for k in range(KH):
                nc.tensor.matmul(
                    ps[:],
                    hstT[:, k, 4 * sb : 4 * sb + 4, :],
                    slab[:, k, :],
                    start=(k == 0),
                    stop=False if has_outb else (k == KH - 1),
                )
            if has_outb:
                nc.tensor.matmul(
                    ps[:], ones_row[:], outb_sb[:, c * VCH : (c + 1) * VCH],
                    start=False, stop=True,
                )
            st = stpool.tile([128, VCH], F32, tag="st")
            nc.scalar.copy(st[:], ps[:])
            # stores on the ACT HWDGE ring; slab loads stay on SP's
            nc.scalar.dma_start(
                d_scores[128 * sb : 128 * (sb + 1), c * VCH : c * VCH + ncols],
                st[:, :ncols],
            )

        # ---- decoder (scores for the first vocab chunks fill step tails) ----
        from collections import deque

        N_INTER = 3
        inter_slabs = []
        for c in range(N_INTER):
            slab = spool.tile([128, KH, VCH], BF16, tag="slab")
            nc.sync.dma_start(slab[:], d_ow[c])
            inter_slabs.append(slab)

        pending = deque()
        gi_t = gipool.tile([128, 3, MG, GCH * TC], F32, tag="gi")
        gi_batch(gi_t, wih_d, xdec[:, :, 0 : GCH * TC], GCH, gib_d)
        for s in range(F):
            g = s // GCH
            gru_step(whh_d, gi_t, s % GCH, ghn_d, hst_write_step=s)
            if s % GCH == 0 and g + 1 < F // GCH:
                gi_next = gipool.tile([128, 3, MG, GCH * TC], F32, tag="gi")
                gi_batch(
                    gi_next, wih_d,
                    xdec[:, :, (g + 1) * GCH * TC : (g + 2) * GCH * TC],
                    GCH, gib_d,
                )
            if s % GCH == GCH - 1 and g + 1 < F // GCH:
                gi_t = gi_next
            # one scores block from an ALREADY-COMPLETE step block per tail
            if pending:
                sb, c = pending.popleft()
                scores_block(sb, c, inter_slabs[c], sb + c)
            if s % 4 == 3:
                pending.extend((s // 4, c) for c in range(N_INTER))
        for sb, c in pending:
            scores_block(sb, c, inter_slabs[c], sb + c)

        # ---- remaining vocab projection ----
        for c in range(N_INTER, NV):
            slab = spool.tile([128, KH, VCH], BF16, tag="slab")
            nc.sync.dma_start(slab[:], d_ow[c])
            for sb in range(F // 4):
                scores_block(sb, c, slab, sb + c)

    nc.compile()
    return nc


def _prep_inputs(token_ctx, char_emb_w, enc_W_ih, enc_W_hh, enc_b_ih, enc_b_hh,
                 dec_W_ih, dec_W_hh, dec_b_ih, dec_b_hh, out_W, out_b,
                 in_sent_token_chars, out_chars):
    """Host-side sharding/layout prep. Returns (in_maps, flags, fixup_info)."""
    tcarr = np.asarray(in_sent_token_chars)[0].reshape(T, C, 3)
    chars = tcarr[:, :, 2]
    xt = tcarr[:, :, 1]
    token_ctx = np.asarray(token_ctx)[0]          # [S, H]
    char_emb_w = np.asarray(char_emb_w)           # [V, E]
    out_chars = np.asarray(out_chars)[0]          # [1 + T*F]

    h0 = token_ctx[xt].mean(axis=1).astype(np.float32)      # [T, H]
    x_enc = char_emb_w[chars]                                # [T, C, E]
    gold = out_chars[1 : 1 + T * F].reshape(T, F)
    c0 = out_chars[0]
    c_in = np.concatenate(
        [np.full((T, 1), c0, dtype=gold.dtype), gold[:, :-1]], axis=1
    )                                                        # [T, F]
    x_dec = char_emb_w[c_in]                                 # [T, F, E]

    # shared (replicated) weight layouts
    whhTe = _to_lhsT_layout(np.asarray(enc_W_hh)).astype(npbf16)
    whhTd = _to_lhsT_layout(np.asarray(dec_W_hh)).astype(npbf16)
    wihTe = _to_lhsT_layout(np.asarray(enc_W_ih)).astype(npbf16)
    wihTd = _to_lhsT_layout(np.asarray(dec_W_ih)).astype(npbf16)
    owpad = np.zeros((VPAD, H), np.float32)
    owpad[:V] = np.asarray(out_W)
    owT = np.ascontiguousarray(
        owpad.reshape(NV, VCH, KH, 128).transpose(0, 3, 2, 1)
    ).astype(npbf16)                                          # [NV,128,KH,VCH]

    def gate_bias(b_ih, b_hh):
        b_ih = np.asarray(b_ih); b_hh = np.asarray(b_hh)
        gib = np.concatenate([b_ih[: 2 * H] + b_hh[: 2 * H], b_ih[2 * H :]])
        ghn = b_hh[2 * H :]
        gib_l = np.ascontiguousarray(gib.reshape(24, 128).T).astype(np.float32)
        ghn_l = np.ascontiguousarray(ghn.reshape(MG, 128).T).astype(np.float32)
        return gib_l, ghn_l, bool(np.any(gib)), bool(np.any(ghn))

    gibE, ghnE, has_gib_e, has_ghn_e = gate_bias(enc_b_ih, enc_b_hh)
    gibD, ghnD, has_gib_d, has_ghn_d = gate_bias(dec_b_ih, dec_b_hh)
    out_b = np.asarray(out_b)
    has_outb = bool(np.any(out_b))
    outb_pad = np.zeros((1, VPAD), npbf16)
    outb_pad[0, :V] = out_b.astype(npbf16)

    flags = (has_gib_e, has_ghn_e, has_gib_d, has_ghn_d, has_outb)

    in_maps = []
    for ci in range(NCORES):
        sl = slice(ci * TC, (ci + 1) * TC)
        h0T = np.ascontiguousarray(
            h0[sl].T.reshape(KH, 128, TC).transpose(1, 0, 2)
        )
        # enc ts = c*TC + t (step-major)
        xe = x_enc[sl].transpose(1, 0, 2).reshape(C * TC, E).astype(np.float32)
        xencT = _cols_layout(xe).astype(npbf16)
        # dec ts = s*TC + t (step-major)
        xd = x_dec[sl].transpose(1, 0, 2).reshape(TS, E).astype(np.float32)
        xdecT = _cols_layout(xd).astype(npbf16)
        m = {
            "h0T": h0T, "xencT": xencT, "xdecT": xdecT,
            "whhTe": whhTe, "whhTd": whhTd, "wihTe": wihTe, "wihTd": wihTd,
            "owT": owT,
        }
        if has_gib_e: m["gibE"] = gibE
        if has_gib_d: m["gibD"] = gibD
        if has_ghn_e: m["ghnE"] = ghnE
        if has_ghn_d: m["ghnD"] = ghnD
        if has_outb: m["outb"] = outb_pad
        in_maps.append(m)

    return in_maps, flags, (gold, c0)


def _eos_fixup(scores, gold, c0):
    """Apply the reference's EOS freeze/pad semantics on the host.
    scores: [T, F, V] (modified in place)."""
    if c0 != EOS and not np.any(gold == EOS):
        return scores
    done0 = c0 == EOS
    for t in range(T):
        hits = np.nonzero(gold[t] == EOS)[0]
        if done0:
            first_done = 0
        elif len(hits):
            first_done = int(hits[0]) + 1
        else:
            continue
        if first_done == 0:
            scores[t, :, :] = 0.0
        elif first_done < F:
            scores[t, first_done:, :] = scores[t, first_done - 1, :]
    return scores


def kernel(**inputs) -> np.ndarray:
    assert int(inputs["max_tokens"]) == T
    assert int(inputs["max_form_len"]) == F
    assert int(inputs["use_teacher_forcing"]) == 1

    in_maps, flags, (gold, c0) = _prep_inputs(
        inputs["token_ctx"], inputs["char_emb_w"],
        inputs["enc_W_ih"], inputs["enc_W_hh"], inputs["enc_b_ih"], inputs["enc_b_hh"],
        inputs["dec_W_ih"], inputs["dec_W_hh"], inputs["dec_b_ih"], inputs["dec_b_hh"],
        inputs["out_W"], inputs["out_b"],
        inputs["in_sent_token_chars"], inputs["out_chars"],
    )

    if flags not in _CACHE:
        _CACHE[flags] = _build_program(flags)
    nc = _CACHE[flags]

    trace = bool(_RUN_OPTS.get("trace"))
    res = run_bass_kernel_spmd(
        nc, in_maps, core_ids=list(range(NCORES)), trace=trace,
        **_RUN_OPTS.get("kwargs", {}),
    )
    _RUN_OPTS["last_result"] = res

    # device rows are step-major per core; reorder to token-major
    slabs = [
        res.results[ci]["scores"].reshape(F, TC, V).transpose(1, 0, 2)
        for ci in range(NCORES)
    ]
    out = np.concatenate(slabs, axis=0)  # [T, F, V]
    out = _eos_fixup(out, gold, c0)
    return np.ascontiguousarray(out.reshape(1, T * F, V))


# knobs used by test.py (harness just calls kernel())
_RUN_OPTS = {"trace": False, "kwargs": {}}
